# revision 1
# baseline (speedup 1.0000x reference)
"""AttentionCritic Trainium2 kernel — 8-core SPMD, no collectives.

Math restructuring (exact up to fp assoc.):
  mask[i,j] = (|x_i-x_j|<=4)&(|y_i-y_j|<=2)&(j>i)
  C = [obs, action];  q/k/v = (C@W{q,k,v}+b)@Wi{q,k,v}+bi
  S_h = q_h k_h^T / 12  (shared over agents); E_h = exp(S_h)  (softmax ratio
  is shift-invariant; |S| small enough that exp is safely fp32)
  D[i,h,j] = sum_k E_h[j,k] mask[i,k];  R = mask/max(D,1e-9)
  W[i,h,k] = mask[i,k] * sum_j R[i,h,j] E_h[j,k]
  ctxs[i,h] = sum_k W[i,h,k] v_h[k]    (= masked sum_j of attention rows)
  h_i = ctxs[i] @ Wo_proj @ W_O + n_i * (bo_proj @ W_O);  Q = V + A - mean(A)

Phase A (through E/E^T/v) replicated on all 8 cores; phase B data-parallel
over agents via a per-core one-hot selector selT (only per-core input).

DMA strategy: host packs weights into contiguous "blob" arrays laid out
exactly as the SBUF tiles (partition-major), one DMA per blob, ordered by
first consumer; a shared HWDGE generator costs ~625ns per DMA instruction
so instruction count matters more than anything; small/broadcast loads go
through the independent SWDGE (gpsimd) path.
"""

import sys

for _p in ("/opt/trn_rl_repo",):
    if _p not in sys.path:
        sys.path.append(_p)

import contextlib

import numpy as np

import concourse.bass as bass
import concourse.bacc as bacc
import concourse.mybir as mybir
from concourse.tile import TileContext
from concourse import bass_utils

N, HID, ACT, NH = 256, 128, 5, 4
D, E, HD = 144, 576, 144
NCORES = 8
SH = N // NCORES  # 32
F32 = mybir.dt.float32
F32R = mybir.dt.float32r
I32 = mybir.dt.int32
SCALE = 1.0 / 12.0
EC = [(0, 128), (128, 128), (256, 128), (384, 128), (512, 64)]
DC = [(0, 128), (128, 16)]
HC = []
for _h in range(NH):
    HC += [(HD * _h, 128), (HD * _h + 128, 16)]

# ---- blob column layouts (host packing must match kernel slicing) ----
# blobA [128]: state0(2) state1(2) hiddenT(256) actionT(256) wenc(16) id128(128)
A_ST0, A_ST1, A_HT, A_AT, A_WENC = 0, 2, 4, 4 + 256, 4 + 512
A_ID = A_WENC + 16
A_COLS = A_ID + 128
# blobB [128]: wqkvB(3*576) baB(4*3) bbB(4*2)
B_WQKV, B_BA, B_BB = 0, 3 * E, 3 * E + 12
B_COLS = B_BB + 8
# blobC/D [128]: padded Wi{q,k} rows 0:512 as 4 blocks of 640
#   col order: 4x[head-main 128] + tailsA[h0:16,pad16,h1:16,pad16] + tailsB[h2,h3]
EP = 640
W4P_COLS = 4 * EP
# blobE [128]: Wiv rows 0:512 as 4 blocks of 576
W4_COLS = 4 * E
QKM = [(0, 128), (128, 128), (256, 128), (384, 128), (512, 64), (576, 64)]
# blobF [128]: wo8big(4*576) wOB(4*144) sel2(2*32) wva128(6)
F_WO8, F_WO, F_SEL, F_WVA = 0, 4 * E, 4 * E + 4 * D, 4 * E + 4 * D + 64
F_COLS = F_WVA + 6
# blobS1 [16]: wqkvS(3*576) bbS(4*2) wvaS(6) benc(1)   (early)
S_WQKV, S_BB, S_WVA, S_BENC = 0, 3 * E, 3 * E + 8, 3 * E + 14
S_COLS = S_BENC + 1
# blobS2 [16]: wo8small(4*576)                          (late)
S2_COLS = 4 * E
# blobT [64]: wiqT(640) wikT(640) wivT(576) wOT(144) baT(3) bbTA(2) bbTB(2)
T_WIQ, T_WIK, T_WIV = 0, EP, 2 * EP
T_WO, T_BA = 2 * EP + E, 2 * EP + E + D
T_BBA, T_BBB = T_BA + 3, T_BA + 5
T_COLS = T_BA + 7


def _build():
    nc = bacc.Bacc(target_bir_lowering=False)

    def dp(name, shape, dtype, isOutput=False):
        return nc.declare_dram_parameter(name, shape, dtype, isOutput)

    blobA_d = dp("blobA", [128, A_COLS], F32)
    blobB_d = dp("blobB", [128, B_COLS], F32)
    wiq_d = dp("blobC", [128, W4P_COLS], F32)
    wik_d = dp("blobD", [128, W4P_COLS], F32)
    wiv_d = dp("blobE", [128, W4_COLS], F32)
    blobF_d = dp("blobF", [128, F_COLS], F32)
    blobS_d = dp("blobS", [16, S_COLS], F32)
    blobS2_d = dp("blobS2", [16, S2_COLS], F32)
    blobT_d = dp("blobT", [64, T_COLS], F32)
    st_d = dp("state", [N, 2], I32)
    biv_d = dp("biv_r", [1, E], F32)
    bo_d = dp("bo_r", [1, E], F32)
    bva_d = dp("bva", [1, 6], F32)
    out_d = dp("out", [SH, ACT], F32, isOutput=True)

    with TileContext(nc) as tc:
        with contextlib.ExitStack() as ctx:
            wp = ctx.enter_context(tc.tile_pool(name="wp", bufs=1))
            pp = ctx.enter_context(tc.tile_pool(name="pp", bufs=7, space="PSUM"))

            def wt(shape, tag, dtype=F32):
                return wp.tile(shape, dtype, tag=tag, name=tag)

            def ps(shape):
                return pp.tile(shape, F32, tag="mm", name="mm")

            dma = nc.sync.dma_start
            gdma = nc.gpsimd.dma_start

            # ---------- blob DMAs, consumer order ----------
            # SP (HWDGE): critical-path weights in dependency order.
            blobA = wt([128, A_COLS], "blobA", F32R)
            dma(out=blobA, in_=blobA_d[:, :].bitcast(F32R))
            blobS = wt([16, S_COLS], "blobS", F32R)
            nc.scalar.dma_start(out=blobS, in_=blobS_d[:, :].bitcast(F32R))
            blobB = wt([128, B_COLS], "blobB", F32R)
            dma(out=blobB, in_=blobB_d[:, :].bitcast(F32R))
            blobT = wt([64, T_COLS], "blobT", F32R)
            nc.scalar.dma_start(out=blobT, in_=blobT_d[:, :].bitcast(F32R))
            wiqB = wt([128, W4P_COLS], "wiqB", F32R)
            dma(out=wiqB[:, 0:2 * EP], in_=wiq_d[:, 0:2 * EP].bitcast(F32R))
            dma(out=wiqB[:, 2 * EP:4 * EP], in_=wiq_d[:, 2 * EP:4 * EP].bitcast(F32R))
            wikB = wt([128, W4P_COLS], "wikB", F32R)
            dma(out=wikB[:, 0:2 * EP], in_=wik_d[:, 0:2 * EP].bitcast(F32R))
            dma(out=wikB[:, 2 * EP:4 * EP], in_=wik_d[:, 2 * EP:4 * EP].bitcast(F32R))
            wivB = wt([128, W4_COLS], "wivB", F32R)
            nc.scalar.dma_start(out=wivB, in_=wiv_d[:, :].bitcast(F32R))
            biv_bc = wt([128, E], "bivbc")
            gdma(out=biv_bc, in_=bass.AP(tensor=biv_d.ap().tensor, offset=0,
                                         ap=[[0, 128], [1, E]]))
            blobF = wt([128, F_COLS], "blobF", F32R)
            nc.scalar.dma_start(out=blobF[:, 0:F_WO],
                                in_=blobF_d[:, 0:F_WO].bitcast(F32R))
            dma(out=blobF[:, F_WO:F_COLS],
                in_=blobF_d[:, F_WO:F_COLS].bitcast(F32R))
            # Pool (SWDGE, independent of HWDGE): small / late loads.
            stfx = wt([1, 256], "stfx", I32)
            gdma(out=stfx, in_=bass.AP(tensor=st_d.ap().tensor, offset=0,
                                       ap=[[1, 1], [2, 256]]))
            stfy = wt([1, 256], "stfy", I32)
            gdma(out=stfy, in_=bass.AP(tensor=st_d.ap().tensor, offset=1,
                                       ap=[[1, 1], [2, 256]]))
            blobS2 = wt([16, S2_COLS], "blobS2", F32R)
            bo_bc = wt([SH, E], "bobc")
            bva_bc = wt([SH, 6], "bvabc")

            # ---------- tile views ----------
            st_i = [blobA[:, A_ST0:A_ST0 + 2].bitcast(F32),
                    blobA[:, A_ST1:A_ST1 + 2].bitcast(F32)]
            hT = blobA[:, A_HT:A_HT + N]
            aT = blobA[:, A_AT:A_AT + N]
            wenc = blobA[:, A_WENC:A_WENC + 16]
            id128 = blobA[:, A_ID:A_ID + 128].bitcast(F32)
            benc = blobS[:, S_BENC:S_BENC + 1].bitcast(F32)
            wqkv = [[blobB[:, B_WQKV + w * E:B_WQKV + (w + 1) * E],
                     blobS[:, S_WQKV + w * E:S_WQKV + (w + 1) * E]]
                    for w in range(3)]
            bq_t = [blobB[:, B_BA + ci * 3:B_BA + ci * 3 + 1].bitcast(F32)
                    for ci in range(4)] + [blobT[:, T_BA:T_BA + 1].bitcast(F32)]
            bk_t = [blobB[:, B_BA + ci * 3 + 1:B_BA + ci * 3 + 2].bitcast(F32)
                    for ci in range(4)] + [blobT[:, T_BA + 1:T_BA + 2].bitcast(F32)]
            bv_t = [blobB[:, B_BA + ci * 3 + 2:B_BA + ci * 3 + 3].bitcast(F32)
                    for ci in range(4)] + [blobT[:, T_BA + 2:T_BA + 3].bitcast(F32)]
            biq_t = [blobB[:, B_BB + 2 * h:B_BB + 2 * h + 1].bitcast(F32)
                     for h in range(NH)] + \
                    [blobT[:, T_BBA:T_BBA + 1].bitcast(F32),
                     blobT[:, T_BBB:T_BBB + 1].bitcast(F32)]
            bik_t = [blobB[:, B_BB + 2 * h + 1:B_BB + 2 * h + 2].bitcast(F32)
                     for h in range(NH)] + \
                    [blobT[:, T_BBA + 1:T_BBA + 2].bitcast(F32),
                     blobT[:, T_BBB + 1:T_BBB + 2].bitcast(F32)]
            wiq_t = [wiqB[:, ci * EP:(ci + 1) * EP] for ci in range(4)] + \
                    [blobT[:, T_WIQ:T_WIQ + EP]]
            wik_t = [wikB[:, ci * EP:(ci + 1) * EP] for ci in range(4)] + \
                    [blobT[:, T_WIK:T_WIK + EP]]
            wiv_t = [wivB[:, ci * E:(ci + 1) * E] for ci in range(4)] + \
                    [blobT[:, T_WIV:T_WIV + E]]
            wo8_t = []
            for h in range(4):
                wo8_t.append(blobF[:, F_WO8 + h * E:F_WO8 + (h + 1) * E])
                wo8_t.append(blobS2[:, h * E:(h + 1) * E])
            wO_t = [blobF[:, F_WO + ci * D:F_WO + (ci + 1) * D] for ci in range(4)] + \
                   [blobT[:, T_WO:T_WO + D]]
            sel_t = [blobF[:, F_SEL:F_SEL + SH].bitcast(F32),
                     blobF[:, F_SEL + SH:F_SEL + 2 * SH].bitcast(F32)]
            wva_t = [blobF[:, F_WVA:F_WVA + 6], blobS[:, S_WVA:S_WVA + 6]]

            # ---------- mask from state (gpsimd; DVE stays free) ----------
            xi = st_i
            stfxf = wt([1, 256], "stfxf")
            stfyf = wt([1, 256], "stfyf")
            nc.gpsimd.tensor_copy(out=stfxf, in_=stfx)
            nc.gpsimd.tensor_copy(out=stfyf, in_=stfy)
            xjf = wt([128, 256], "xjf")
            yjf = wt([128, 256], "yjf")
            nc.gpsimd.partition_broadcast(xjf, stfxf)
            nc.gpsimd.partition_broadcast(yjf, stfyf)

            mask_t = []
            for c in range(2):
                bx = wt([128, 256], f"bx{c}")
                by = wt([128, 256], f"by{c}")
                bx2 = wt([128, 256], f"bx2{c}")
                by2 = wt([128, 256], f"by2{c}")
                nc.gpsimd.tensor_scalar(bx, xjf, xi[c][:, 0:1], None,
                                        mybir.AluOpType.subtract)
                nc.gpsimd.tensor_scalar(by, yjf, xi[c][:, 1:2], None,
                                        mybir.AluOpType.subtract)
                nc.gpsimd.tensor_scalar(bx2, bx, -4.0, None, mybir.AluOpType.is_ge)
                nc.gpsimd.tensor_scalar(bx, bx, 4.0, None, mybir.AluOpType.is_le)
                nc.gpsimd.tensor_scalar(by2, by, -2.0, None, mybir.AluOpType.is_ge)
                nc.gpsimd.tensor_scalar(by, by, 2.0, None, mybir.AluOpType.is_le)
                nc.gpsimd.tensor_tensor(bx, bx, bx2, mybir.AluOpType.mult)
                nc.gpsimd.tensor_tensor(by, by, by2, mybir.AluOpType.mult)
                prox = wt([128, 256], f"prox{c}")
                nc.gpsimd.tensor_tensor(prox, bx, by, mybir.AluOpType.mult)
                mk = wt([128, 256], f"mask{c}")
                nc.gpsimd.affine_select(out=mk, in_=prox, pattern=[[1, 256]],
                                        compare_op=mybir.AluOpType.is_gt,
                                        fill=0.0, base=-c * 128,
                                        channel_multiplier=-1)
                mask_t.append(mk)

            # ---------- obs^T = W_enc^T @ hidden^T + b_enc ----------
            obsT = wt([16, N], "obsT", F32R)
            p0 = ps([16, N])
            nc.tensor.matmul(p0, wenc, hT, start=True, stop=True)
            nc.vector.tensor_scalar(obsT, p0, benc, None, mybir.AluOpType.add)
            CT = [aT, obsT]

            # ---------- t^T = W^T C'^T + b (x3) ----------
            def proj_t(wtiles, btiles, tag):
                outs = []
                for mi, (ms, ml) in enumerate(EC):
                    p = ps([ml, N])
                    for ci in range(2):
                        nc.tensor.matmul(p, wtiles[ci][:, ms:ms + ml], CT[ci],
                                         start=(ci == 0), stop=(ci == 1))
                    t = wt([ml, N], f"{tag}{mi}", F32R)
                    nc.vector.tensor_scalar(t, p, btiles[mi], None,
                                            mybir.AluOpType.add)
                    outs.append(t)
                return outs

            tqT = proj_t(wqkv[0], bq_t, "tqT")
            tkT = proj_t(wqkv[1], bk_t, "tkT")
            tvT = proj_t(wqkv[2], bv_t, "tvT")

            # ---------- q^T / k^T (HC-tiled so head rows start at part 0) ----
            def proj_qk(wi_t, tT, bt, tag, eng):
                outs = []
                for mi, (ms, ml) in enumerate(QKM):
                    p = ps([ml, N])
                    for ci, (cs, cl) in enumerate(EC):
                        nc.tensor.matmul(p, wi_t[ci][:, ms:ms + ml], tT[ci],
                                         start=(ci == 0), stop=(ci == len(EC) - 1))
                    t = wt([ml, N], f"{tag}{mi}", F32R)
                    if eng == "act":
                        nc.scalar.activation(t, p,
                                             mybir.ActivationFunctionType.Identity,
                                             bias=bt[mi], scale=1.0)
                    else:
                        nc.vector.tensor_scalar(t, p, bt[mi], None,
                                                mybir.AluOpType.add)
                    outs.append(t)
                return outs

            qT = proj_qk(wiq_t, tqT, biq_t, "qT", "dve")
            kT = proj_qk(wik_t, tkT, bik_t, "kT", "dve")

            # ---------- S_h, S_h^T -> E_h, E_h^T ----------
            E_t = [[None, None] for _ in range(NH)]
            ET_t = [[None, None] for _ in range(NH)]
            for h in range(NH):
                hs = [(h, 0, 128), (4 + h // 2, 32 * (h % 2), 32)]
                for mj in range(2):
                    pS = ps([128, N])
                    pST = ps([128, N])
                    for ci, (ti, rs, rl) in enumerate(hs):
                        st_, sp = (ci == 0), (ci == len(hs) - 1)
                        nc.tensor.matmul(
                            pS, qT[ti][rs:rs + rl, mj * 128:(mj + 1) * 128],
                            kT[ti][rs:rs + rl, :], start=st_, stop=sp)
                        nc.tensor.matmul(
                            pST, kT[ti][rs:rs + rl, mj * 128:(mj + 1) * 128],
                            qT[ti][rs:rs + rl, :], start=st_, stop=sp)
                    Eh = wt([128, N], f"E{h}_{mj}", F32R)
                    ETh = wt([128, N], f"ET{h}_{mj}", F32R)
                    nc.scalar.activation(Eh, pS, mybir.ActivationFunctionType.Exp,
                                         scale=SCALE)
                    nc.scalar.activation(ETh, pST,
                                         mybir.ActivationFunctionType.Exp,
                                         scale=SCALE)
                    E_t[h][mj] = Eh
                    ET_t[h][mj] = ETh

            gdma(out=blobS2, in_=blobS2_d[:, :].bitcast(F32R))
            gdma(out=bo_bc, in_=bass.AP(tensor=bo_d.ap().tensor, offset=0,
                                        ap=[[0, SH], [1, E]]))
            gdma(out=bva_bc, in_=bass.AP(tensor=bva_d.ap().tensor, offset=0,
                                         ap=[[0, SH], [1, 6]]))
            # ---------- v = (t_v^T)^T Wiv + biv  [n, E] ----------
            v_t = []
            for nt in range(2):
                vt = wt([128, E], f"v{nt}", F32R)
                for ns, nl in ((0, 288), (288, 288)):
                    p = ps([128, nl])
                    for ci, (cs, cl) in enumerate(EC):
                        nc.tensor.matmul(
                            p, tvT[ci][:, nt * 128:(nt + 1) * 128],
                            wiv_t[ci][:, ns:ns + nl],
                            start=(ci == 0), stop=(ci == len(EC) - 1))
                    nc.vector.tensor_tensor(vt[:, ns:ns + nl], p,
                                            biv_bc[:, ns:ns + nl],
                                            mybir.AluOpType.add)
                v_t.append(vt)

            # ---------- phase B: this core's 32 agents (agents on free dim) --
            mcT = []
            for km in range(2):
                p = ps([128, SH])
                for c in range(2):
                    nc.tensor.matmul(
                        p, mask_t[c][:, km * 128:(km + 1) * 128],
                        sel_t[c], start=(c == 0), stop=(c == 1))
                t = wt([128, SH], f"mcT{km}", F32R)
                nc.any.tensor_copy(out=t, in_=p)
                mcT.append(t)
            ones_t = wt([128, 1], "ones_t")
            nc.vector.memset(ones_t, 1.0)
            pn = ps([SH, 1])
            for c in range(2):
                nc.tensor.matmul(pn, mcT[c].bitcast(F32), ones_t,
                                 start=(c == 0), stop=(c == 1))
            n_i = wt([SH, 1], "n_i")
            nc.any.tensor_copy(out=n_i, in_=pn)

            ctxT8 = [None] * 8
            for hg in (0, 2):
                RTg, WTg = {}, {}
                for h in (hg, hg + 1):
                    for jm in range(2):
                        p = ps([128, SH])
                        for kc in range(2):
                            nc.tensor.matmul(
                                p, ET_t[h][kc][:, jm * 128:(jm + 1) * 128],
                                mcT[kc], start=(kc == 0), stop=(kc == 1))
                        rt = wt([128, SH], f"RT{h}_{jm}", F32R)
                        nc.vector.tensor_scalar(rt, p, 1e-9, None,
                                                mybir.AluOpType.max)
                        with nc.allow_low_precision(reason="fp32r attn renorm"):
                            nc.vector.reciprocal(rt, rt)
                        nc.vector.tensor_tensor(rt, rt, mcT[jm].bitcast(F32),
                                                mybir.AluOpType.mult)
                        RTg[(h, jm)] = rt
                for h in (hg, hg + 1):
                    for km in range(2):
                        p = ps([128, SH])
                        for jc in range(2):
                            nc.tensor.matmul(
                                p, E_t[h][jc][:, km * 128:(km + 1) * 128],
                                RTg[(h, jc)], start=(jc == 0), stop=(jc == 1))
                        wtl = wt([128, SH], f"WT{h}_{km}", F32R)
                        nc.vector.tensor_tensor(wtl, p, mcT[km].bitcast(F32),
                                                mybir.AluOpType.mult)
                        WTg[(h, km)] = wtl
                for h in (hg, hg + 1):
                    for dm, (ds, dl) in enumerate([(0, 128), (128, 16)]):
                        p = ps([dl, SH])
                        for kc in range(2):
                            nc.tensor.matmul(
                                p, v_t[kc][:, HD * h + ds:HD * h + ds + dl],
                                WTg[(h, kc)], start=(kc == 0), stop=(kc == 1))
                        t = wt([dl, SH], f"cT{2 * h + dm}", F32R)
                        nc.any.tensor_copy(out=t, in_=p)
                        ctxT8[2 * h + dm] = t

            # ho = ctx @ Wo + n_i * bo   [32, 576]
            ho_sb = wt([SH, E], "hosb")
            for ns, nl in ((0, 288), (288, 288)):
                p = ps([SH, nl])
                for ci in range(8):
                    nc.tensor.matmul(p, ctxT8[ci], wo8_t[ci][:, ns:ns + nl],
                                     start=(ci == 0), stop=(ci == 7))
                nc.vector.scalar_tensor_tensor(
                    out=ho_sb[:, ns:ns + nl], in0=bo_bc[:, ns:ns + nl],
                    scalar=n_i, in1=p,
                    op0=mybir.AluOpType.mult, op1=mybir.AluOpType.add)

            # ho^T via PE transpose; hfeat^T = W_O^T ho^T; VA; dueling Q
            hoT = []
            for ci, (cs, cl) in enumerate(EC):
                p = ps([cl, SH])
                nc.tensor.transpose(p, ho_sb[:, cs:cs + cl], id128[0:SH, 0:SH])
                t = wt([cl, SH], f"hoT{ci}", F32R)
                nc.any.tensor_copy(out=t, in_=p)
                hoT.append(t)
            hfT = []
            for mi, (ms, ml) in enumerate(DC):
                p = ps([ml, SH])
                for ci, (cs, cl) in enumerate(EC):
                    nc.tensor.matmul(p, wO_t[ci][:, ms:ms + ml], hoT[ci],
                                     start=(ci == 0), stop=(ci == len(EC) - 1))
                t = wt([ml, SH], f"hfT{mi}", F32R)
                nc.any.tensor_copy(out=t, in_=p)
                hfT.append(t)
            pVA = ps([SH, 6])
            for ci in range(2):
                nc.tensor.matmul(pVA, hfT[ci], wva_t[ci],
                                 start=(ci == 0), stop=(ci == 1))
            VA = wt([SH, 6], "VA")
            nc.vector.tensor_tensor(VA, pVA, bva_bc, mybir.AluOpType.add)
            sA = wt([SH, 1], "sA")
            nc.vector.reduce_sum(sA, VA[:, 1:6], axis=mybir.AxisListType.X)
            vm = wt([SH, 1], "vm")
            nc.vector.scalar_tensor_tensor(out=vm, in0=sA, scalar=-0.2,
                                           in1=VA[:, 0:1],
                                           op0=mybir.AluOpType.mult,
                                           op1=mybir.AluOpType.add)
            Q_sb = wt([SH, ACT], "Qsb")
            nc.vector.tensor_scalar(Q_sb, VA[:, 1:6], vm, None,
                                    mybir.AluOpType.add)
            nc.gpsimd.dma_start(out=out_d[:, :], in_=Q_sb)

    nc.compile()
    return nc


_NC_CACHE = {}


def _make_in_maps(inputs):
    f32 = np.float32
    g = lambda k: np.ascontiguousarray(np.asarray(inputs[k]), dtype=f32)
    perm = lambda w: np.concatenate([w[16:144], w[0:16]], axis=0)

    hidden, action = g("hidden_state_n"), g("action_n")
    state = np.ascontiguousarray(np.asarray(inputs["state_n"]), dtype=np.int32)
    Wq, Wk, Wv = perm(g("Wq")), perm(g("Wk")), perm(g("Wv"))
    biasA = np.stack([g("bq"), g("bk"), g("bv")], axis=1)          # [576,3]
    biasB = np.stack([g("biq"), g("bik")], axis=1)                 # [576,2]
    Wiq, Wik, Wiv = g("Wiq"), g("Wik"), g("Wiv")

    def padqk(w):  # [X,576] -> [X,640]: 4 head-mains + 2 padded tail blocks
        mains = [w[:, 144 * h:144 * h + 128] for h in range(4)]
        z = np.zeros((w.shape[0], 16), f32)
        tails = [np.concatenate([w[:, 144 * h + 128:144 * h + 144], z,
                                 w[:, 144 * (h + 1) + 128:144 * (h + 1) + 144],
                                 z], axis=1) for h in (0, 2)]
        return np.concatenate(mains + tails, axis=1)

    WiqP, WikP = padqk(Wiq), padqk(Wik)
    # biasB mains [144h:144h+128] -> rows 128h..; tails padded like padqk
    bbz = np.zeros((16, 2), f32)
    bbTA = np.concatenate([biasB[128:144], bbz, biasB[272:288], bbz], axis=0)
    bbTB = np.concatenate([biasB[416:432], bbz, biasB[560:576], bbz], axis=0)
    Wo, W_O = g("Wo_proj"), g("W_O")
    Wva = np.concatenate([g("W_val").reshape(D, 1),
                          g("W_adv").reshape(D, ACT)], axis=1)     # [144,6]
    eye = np.eye(N, dtype=f32)

    def blocks128(w):   # rows 0:512 -> [128, 4*cols]
        return w[0:512].reshape(4, 128, -1).transpose(1, 0, 2).reshape(128, -1)

    def hblocks(w, rows, r0):  # 144-row blocks -> [rows, 4*cols]
        return np.concatenate([w[144 * h + r0:144 * h + r0 + rows]
                               for h in range(4)], axis=1)

    state_f = state.astype(f32)
    blobA = np.concatenate([
        state_f[0:128], state_f[128:256],
        np.ascontiguousarray(hidden.T), np.ascontiguousarray(action.T),
        g("W_enc"), np.eye(128, dtype=f32)], axis=1)
    blobB = np.concatenate([
        np.concatenate([Wq[0:128], Wk[0:128], Wv[0:128]], axis=1),
        blocks128(biasA), hblocks(biasB, 128, 0)], axis=1)
    blobF = np.concatenate([
        hblocks(Wo, 128, 0), blocks128(W_O),
        np.zeros((128, 2 * SH), f32), Wva[0:128]], axis=1)
    blobS = np.concatenate([
        np.concatenate([Wq[128:144], Wk[128:144], Wv[128:144]], axis=1),
        hblocks(biasB, 16, 128),
        Wva[128:144], g("b_enc").reshape(16, 1)], axis=1)
    blobS2 = hblocks(Wo, 16, 128)
    blobT = np.concatenate([
        WiqP[512:576], WikP[512:576], Wiv[512:576], W_O[512:576],
        biasA[512:576], bbTA, bbTB], axis=1)

    shared = {
        "blobA": np.ascontiguousarray(blobA, dtype=f32),
        "blobB": np.ascontiguousarray(blobB, dtype=f32),
        "blobC": np.ascontiguousarray(blocks128(WiqP), dtype=f32),
        "blobD": np.ascontiguousarray(blocks128(WikP), dtype=f32),
        "blobE": np.ascontiguousarray(blocks128(Wiv), dtype=f32),
        "blobS": np.ascontiguousarray(blobS, dtype=f32),
        "blobS2": np.ascontiguousarray(blobS2, dtype=f32),
        "blobT": np.ascontiguousarray(blobT, dtype=f32),
        "state": state,
        "biv_r": g("biv").reshape(1, E),
        "bo_r": g("bo_proj").reshape(1, E),
        "bva": np.concatenate([g("b_val").reshape(1),
                               g("b_adv").reshape(ACT)]).reshape(1, 6)
        .astype(f32),
    }
    in_maps = []
    for c in range(NCORES):
        sel = eye[:, c * SH:(c + 1) * SH]              # [256, 32]
        selpack = np.concatenate([sel[0:128], sel[128:256]], axis=1)  # [128,64]
        bF = blobF.copy()
        bF[:, F_SEL:F_SEL + 2 * SH] = selpack
        m = dict(shared)
        m["blobF"] = np.ascontiguousarray(bF, dtype=f32)
        in_maps.append(m)
    return in_maps


def kernel(**inputs):
    if "nc" not in _NC_CACHE:
        _NC_CACHE["nc"] = _build()
    nc = _NC_CACHE["nc"]
    in_maps = _make_in_maps(inputs)
    res = bass_utils.run_bass_kernel_spmd(nc, in_maps, core_ids=list(range(NCORES)))
    return np.concatenate([res.results[c]["out"] for c in range(NCORES)], axis=0)



# revision 3
# speedup vs baseline: 4.0223x; 4.0223x over previous
"""AttentionCritic Trainium2 kernel — 8-core SPMD, head/query-half sharded.

Math restructuring (exact up to fp assoc.):
  mask[i,j] = (|x_i-x_j|<=4)&(|y_i-y_j|<=2)&(j>i)          (host, from int state)
  C = [obs, action];  obs = h @ W_enc + b_enc
  q = C @ (Wq Wiq) + (bq Wiq + biq)   (two-stage projections folded on host)
  S_h = q_h k_h^T / 12;  E_h = exp(S_h)  (softmax ratio is shift-invariant;
  |S| small enough that exp is safely fp32 — validated by prior baseline)
  D[i,h,j] = sum_k E_h[j,k] mask[i,k];  R = mask^T/max(D,1e-9)
  W[i,h,k] = mask[i,k] * sum_j R[j,i] E_h[j,k]
  ctxs[i,h] = sum_k W[i,h,k] v_h[k]
  Q_i = sum_h ctxs[i,h] @ W_out_h + n_i*c1 + c2
  where W_out = (Wo_proj W_O) Wdueling [576,5] folds the entire output side
  (out-proj, W_O, masked j-sum, dueling V/A head) into one tiny GEMM; the
  n_i*c1+c2 affine part is added on host (n_i = mask row sums).

Sharding: core c handles (head h=c//2, query-half jm=c%2). Everything after
exp is linear in j and h, so each core emits a partial Q^T [5,256] over ALL
256 agents (free dim 256 keeps fp32r matmuls at full 1 cyc/row rate) and the
host sums the 8 partials. The j-half selection is uniform across cores: the
per-core input packing rotates the agent axis by 128*jm, so slice [0:128]
is always "my" j-half.

Per-core cost: ~29 matmuls, ~790KB of DMA (vs 236 matmuls / 7.7MB for the
agent-sharded unfolded version).
"""

import sys

for _p in ("/opt/trn_rl_repo",):
    if _p not in sys.path:
        sys.path.append(_p)

import contextlib

import numpy as np

import concourse.bass as bass
import concourse.bacc as bacc
import concourse.mybir as mybir
from concourse.tile import TileContext
from concourse import bass_utils

N, HID, ACT, NH = 256, 128, 5, 4
D, E, HD = 144, 576, 144
NCORES = 8
F32 = mybir.dt.float32
F32R = mybir.dt.float32r
SCALE = 1.0 / 12.0

# blob1 [128, B1_COLS] column layout (host packing must match kernel slicing)
B1_WENC = 0            # W_enc [128,16]
B1_HT = 16             # hidden^T perm [128,256]
B1_AT = 272            # action^T perm [128,256]
B1_WQA = 528           # Wq_eff[16:144, head] [128,144]
B1_WKA = 672           # Wk_eff[16:144, head] [128,144]
B1A_COLS = 816
B1_WVA = 816           # Wv_eff[16:144, head] [128,144]
B1_MT0 = 960           # mask^T perm rows 0:128 [128,256]
B1_MT1 = 1216          # mask^T perm rows 128:256 [128,256]
B1_WOM = 1472          # W_out head main rows [128,5]
B1_COLS = 1477
# blob2 [17, B2_COLS]: rows 0:16 = obs-feature rows, row 16 = bias row
B2_WQB, B2_WKB, B2_WVB = 0, 144, 288
B2_BENC = 432          # b_enc in rows 0:16 of col 432
B2_WOT = 433           # W_out head tail rows [16,5] (row 16 zero)
B2_COLS = 438


def _build():
    nc = bacc.Bacc(target_bir_lowering=False)

    def dp(name, shape, dtype, isOutput=False):
        return nc.declare_dram_parameter(name, shape, dtype, isOutput)

    b1a_d = dp("blob1a", [128, B1A_COLS], F32)
    b1b_d = dp("blob1b", [128, B1_COLS - B1A_COLS], F32)
    b2_d = dp("blob2", [17, B2_COLS], F32)
    out_d = dp("out", [5, N], F32, isOutput=True)

    with TileContext(nc) as tc:
        with contextlib.ExitStack() as ctx:
            wp = ctx.enter_context(tc.tile_pool(name="wp", bufs=1))
            pp = ctx.enter_context(tc.tile_pool(name="pp", bufs=8, space="PSUM"))

            def wt(shape, tag, dtype=F32R):
                return wp.tile(shape, dtype, tag=tag, name=tag)

            def ps(shape):
                return pp.tile(shape, F32, tag="mm", name="mm")

            # ---------- DMAs on three independent queues ----------
            b1a = wt([128, B1A_COLS], "b1a")
            nc.sync.dma_start(out=b1a, in_=b1a_d[:, :].bitcast(F32R))
            b1b = wt([128, B1_COLS - B1A_COLS], "b1b")
            nc.scalar.dma_start(out=b1b, in_=b1b_d[:, :].bitcast(F32R))
            b2 = wt([17, B2_COLS], "b2")
            nc.gpsimd.dma_start(out=b2, in_=b2_d[:, :].bitcast(F32R))

            wenc = b1a[:, B1_WENC:B1_WENC + 16]
            hTp = b1a[:, B1_HT:B1_HT + N]
            aTp = b1a[:, B1_AT:B1_AT + N]
            WqA = b1a[:, B1_WQA:B1_WQA + 144]
            WkA = b1a[:, B1_WKA:B1_WKA + 144]
            WvA = b1b[:, B1_WVA - B1A_COLS:B1_WVA - B1A_COLS + 144]
            mT = [b1b[:, B1_MT0 - B1A_COLS:B1_MT0 - B1A_COLS + N],
                  b1b[:, B1_MT1 - B1A_COLS:B1_MT1 - B1A_COLS + N]]
            WoM = b1b[:, B1_WOM - B1A_COLS:B1_WOM - B1A_COLS + 5]
            WqB = b2[:, B2_WQB:B2_WQB + 144]
            WkB = b2[:, B2_WKB:B2_WKB + 144]
            WvB = b2[:, B2_WVB:B2_WVB + 144]
            benc = b2[0:16, B2_BENC:B2_BENC + 1].bitcast(F32)
            WoT = b2[0:16, B2_WOT:B2_WOT + 5]

            # ---------- C^T tail tile: obs rows + ones row ----------
            # (memset whole 32-partition tile to 1.0 — engine writes must
            # start on a 32-partition boundary — then overwrite rows 0:16
            # with obs; row 16 keeps the 1.0 bias row.)
            ctT_full = wt([32, N], "ctT")
            nc.vector.memset(ctT_full[:, :].bitcast(F32), 1.0)
            pObs = ps([16, N])
            nc.tensor.matmul(pObs, wenc, hTp, start=True, stop=True)
            nc.scalar.activation(ctT_full[0:16, :], pObs,
                                 mybir.ActivationFunctionType.Identity,
                                 bias=benc, scale=1.0)
            ctT = ctT_full[0:17, :]

            # ---------- q^T, k^T for this head: [128,256] + [16,256] ----------
            def proj_qk(W_A, W_B, tag, eng):
                pm = ps([128, N])
                nc.tensor.matmul(pm, W_A[:, 0:128], aTp, start=True, stop=False)
                nc.tensor.matmul(pm, W_B[:, 0:128], ctT, start=False, stop=True)
                tm = wt([128, N], f"{tag}m")
                eng_copy(eng, tm, pm)
                pt = ps([16, N])
                nc.tensor.matmul(pt, W_A[:, 128:144], aTp, start=True, stop=False)
                nc.tensor.matmul(pt, W_B[:, 128:144], ctT, start=False, stop=True)
                tt = wt([16, N], f"{tag}t")
                eng_copy(eng, tt, pt)
                return tm, tt

            def eng_copy(eng, out, in_):
                if eng == "act":
                    nc.scalar.activation(out, in_,
                                         mybir.ActivationFunctionType.Identity,
                                         scale=1.0)
                else:
                    nc.vector.tensor_copy(out=out, in_=in_)

            qTm, qTt = proj_qk(WqA, WqB, "qT", "act")
            kTm, kTt = proj_qk(WkA, WkB, "kT", "dve")

            # ---------- S = q_jhalf k^T, E = exp(S/12) ----------
            pS = ps([128, N])
            nc.tensor.matmul(pS, qTm[:, 0:128], kTm, start=True, stop=False)
            nc.tensor.matmul(pS, qTt[:, 0:128], kTt, start=False, stop=True)
            Et = wt([128, N], "Et")
            nc.scalar.activation(Et, pS, mybir.ActivationFunctionType.Exp,
                                 scale=SCALE)

            # ---------- S^T chunks, E^T = exp(S^T/12) ----------
            ET = []
            for kc in range(2):
                pT = ps([128, 128])
                nc.tensor.matmul(pT, kTm[:, kc * 128:(kc + 1) * 128],
                                 qTm[:, 0:128], start=True, stop=False)
                nc.tensor.matmul(pT, kTt[:, kc * 128:(kc + 1) * 128],
                                 qTt[:, 0:128], start=False, stop=True)
                t = wt([128, 128], f"ET{kc}")
                nc.scalar.activation(t, pT, mybir.ActivationFunctionType.Exp,
                                     scale=SCALE)
                ET.append(t)

            # ---------- v = C @ Wv_eff_h : [128,144] x2 n-chunks ----------
            v_t = []
            for kc in range(2):
                pv = ps([128, HD])
                nc.tensor.matmul(pv, aTp[:, kc * 128:(kc + 1) * 128], WvA,
                                 start=True, stop=False)
                nc.tensor.matmul(pv, ctT[:, kc * 128:(kc + 1) * 128], WvB,
                                 start=False, stop=True)
                t = wt([128, HD], f"v{kc}")
                nc.vector.tensor_copy(out=t, in_=pv)
                v_t.append(t)

            # ---------- D = E^T-contract with mask^T; R = mask^T/max(D,eps) --
            pD = ps([128, N])
            nc.tensor.matmul(pD, ET[0], mT[0], start=True, stop=False)
            nc.tensor.matmul(pD, ET[1], mT[1], start=False, stop=True)
            R = wt([128, N], "R")
            nc.vector.tensor_scalar(R, pD, 1e-9, None, mybir.AluOpType.max)
            with nc.allow_low_precision(reason="fp32r attn renorm"):
                nc.vector.reciprocal(R, R)
            nc.vector.tensor_tensor(R, R, mT[0].bitcast(F32),
                                    mybir.AluOpType.mult)

            # ---------- W[k,i] = mask^T * (E^T-partial over my j-half) ------
            Wt = []
            for kc in range(2):
                pW = ps([128, N])
                nc.tensor.matmul(pW, Et[:, kc * 128:(kc + 1) * 128], R,
                                 start=True, stop=True)
                t = wt([128, N], f"W{kc}")
                nc.vector.tensor_tensor(t, pW, mT[kc].bitcast(F32),
                                        mybir.AluOpType.mult)
                Wt.append(t)

            # ---------- ctx^T = v^T-contract: [128,256] + [16,256] ----------
            pCm = ps([128, N])
            nc.tensor.matmul(pCm, v_t[0][:, 0:128], Wt[0], start=True, stop=False)
            nc.tensor.matmul(pCm, v_t[1][:, 0:128], Wt[1], start=False, stop=True)
            cTm = wt([128, N], "cTm")
            nc.scalar.activation(cTm, pCm, mybir.ActivationFunctionType.Identity,
                                 scale=1.0)
            pCt = ps([16, N])
            nc.tensor.matmul(pCt, v_t[0][:, 128:144], Wt[0], start=True, stop=False)
            nc.tensor.matmul(pCt, v_t[1][:, 128:144], Wt[1], start=False, stop=True)
            cTt = wt([16, N], "cTt")
            nc.vector.tensor_copy(out=cTt, in_=pCt)

            # ---------- partial Q^T = W_out_h^T ctx^T : [5,256] ----------
            pQ = ps([5, N])
            nc.tensor.matmul(pQ, WoM, cTm, start=True, stop=False)
            nc.tensor.matmul(pQ, WoT, cTt, start=False, stop=True)
            Qsb = wt([5, N], "Qsb", F32)
            nc.vector.tensor_copy(out=Qsb, in_=pQ)
            nc.gpsimd.dma_start(out=out_d[:, :], in_=Qsb)

    nc.compile()
    return nc


_NC_CACHE = {}


def _make_in_maps(inputs):
    f32 = np.float32
    g = lambda k: np.asarray(inputs[k], dtype=np.float64)

    hidden = np.asarray(inputs["hidden_state_n"], dtype=f32)
    action = np.asarray(inputs["action_n"], dtype=f32)
    state = np.asarray(inputs["state_n"]).astype(np.int64)

    # host-side weight folding (float64 for exactness, cast to f32)
    Wq_eff = g("Wq") @ g("Wiq")
    bq_eff = g("bq") @ g("Wiq") + g("biq")
    Wk_eff = g("Wk") @ g("Wik")
    bk_eff = g("bk") @ g("Wik") + g("bik")
    Wv_eff = g("Wv") @ g("Wiv")
    bv_eff = g("bv") @ g("Wiv") + g("biv")
    Wo_eff = g("Wo_proj") @ g("W_O")          # [576,144]
    bo_eff = g("bo_proj") @ g("W_O")          # [144]
    # dueling fold: Q = h @ W_Q + b_Q
    W_adv = g("W_adv")
    W_Q = (g("W_val") @ np.ones((1, ACT)) + W_adv
           - (W_adv @ np.ones((ACT, ACT))) / ACT)              # [144,5]
    b_adv = g("b_adv")
    b_Q = g("b_val")[0] + b_adv - b_adv.mean()                 # [5]
    W_out = (Wo_eff @ W_Q).astype(f32)                         # [576,5]
    c1 = (bo_eff @ W_Q).astype(f32)                            # [5]
    c2 = b_Q.astype(f32)                                       # [5]

    # mask from int state (host): mask[i,j] = j observed by i
    dx = np.abs(state[:, None, 0] - state[None, :, 0])
    dy = np.abs(state[:, None, 1] - state[None, :, 1])
    upper = np.arange(N)[None, :] > np.arange(N)[:, None]
    mask = ((dx <= 4) & (dy <= 2) & upper).astype(f32)         # [N,N]
    n_i = mask.sum(axis=1)                                     # [N]
    maskT = np.ascontiguousarray(mask.T)                       # [j,i]

    W_enc = np.asarray(inputs["W_enc"], dtype=f32)             # [128,16]
    b_enc = np.asarray(inputs["b_enc"], dtype=f32)             # [16]
    hT = np.ascontiguousarray(hidden.T)                        # [128,256]
    aT = np.ascontiguousarray(action.T)

    in_maps = []
    for c in range(NCORES):
        h, jm = c // 2, c % 2
        perm = np.roll(np.arange(N), -jm * 128)
        cols = slice(144 * h, 144 * h + 144)
        WqAh = Wq_eff[16:144, cols].astype(f32)
        WkAh = Wk_eff[16:144, cols].astype(f32)
        WvAh = Wv_eff[16:144, cols].astype(f32)
        WqBh = np.vstack([Wq_eff[0:16, cols], bq_eff[None, cols]]).astype(f32)
        WkBh = np.vstack([Wk_eff[0:16, cols], bk_eff[None, cols]]).astype(f32)
        WvBh = np.vstack([Wv_eff[0:16, cols], bv_eff[None, cols]]).astype(f32)
        mTp = maskT[perm, :]
        b1a = np.concatenate([W_enc, hT[:, perm], aT[:, perm], WqAh, WkAh],
                             axis=1)
        b1b = np.concatenate([WvAh, mTp[0:128], mTp[128:256],
                              W_out[144 * h:144 * h + 128]], axis=1)
        b2 = np.concatenate([
            WqBh, WkBh, WvBh,
            np.concatenate([b_enc.reshape(16, 1), np.zeros((1, 1), f32)]),
            np.concatenate([W_out[144 * h + 128:144 * h + 144],
                            np.zeros((1, 5), f32)])], axis=1)
        in_maps.append({
            "blob1a": np.ascontiguousarray(b1a, dtype=f32),
            "blob1b": np.ascontiguousarray(b1b, dtype=f32),
            "blob2": np.ascontiguousarray(b2, dtype=f32),
        })
    return in_maps, n_i, c1, c2


def kernel(**inputs):
    if "nc" not in _NC_CACHE:
        _NC_CACHE["nc"] = _build()
    nc = _NC_CACHE["nc"]
    in_maps, n_i, c1, c2 = _make_in_maps(inputs)
    res = bass_utils.run_bass_kernel_spmd(nc, in_maps, core_ids=list(range(NCORES)))
    QT = np.zeros((ACT, N), np.float32)
    for c in range(NCORES):
        QT += res.results[c]["out"]
    Q = QT.T + n_i[:, None] * c1[None, :] + c2[None, :]
    return Q.astype(np.float32)


# revision 5
# speedup vs baseline: 4.7594x; 1.1832x over previous
"""AttentionCritic Trainium2 kernel — 8-core SPMD, head/query-half sharded.

Math restructuring (exact up to fp assoc.):
  mask[i,j] = (|x_i-x_j|<=4)&(|y_i-y_j|<=2)&(j>i)          (host, from int state)
  C' = [act(128), obs(16), 1]  (ones row folds all biases)
  S_h = C' G' C'^T / 12 with G' = Aq Ak^T host-folded [145,145]
        (Aq = [Wq_eff rows | bq_eff] etc. — the two-stage reference
        projections (C@Wq+bq)@Wiq+biq are first folded to single eff mats)
  T1 = C' G' computed X-form: T1^T = G''^T [h;a]^T + bT1 (G'' absorbs the
        obs encoder for T1 only; bT1 = ones-row of G', added in the
        PSUM->SBUF copy as a per-partition bias)
  E_h = exp(S_h)  (softmax ratio is shift-invariant; |S| small enough that
        exp is safely fp32 — validated on this data by earlier versions)
  D[j,i] = sum_k E[j,k] mask[i,k];  R = mask^T/max(D,1e-9)  (approx recip)
  W[k,i] = mask[i,k] * sum_j E[j,k] R[j,i]
  u = C' @ (Av W_out_h)  [256,5]  (v/ctx/out-proj fold: Q_p = sum_k W[k,i] u[k,a])
  Q = sum_cores Q_p^T + n_i*c1 + c2  (host; W_out = Wo_eff @ Wdueling [576,5])

Sharding: core c handles (head h=c//2, query-half jm=c%2). Everything after
exp is linear in j and h, so each core emits a partial Q^T [5,256] over ALL
256 agents (free dim 256 keeps fp32r matmuls at the fast 1 cyc/row rate) and
the host sums the 8 partials. The j-half selection is uniform across cores:
the per-core input packing rotates the agent axis by 128*jm, so slice
[0:128] is always "my" j-half.

Per-core: 19 matmuls / ~3.9K stream rows, ~700KB DMA.
"""

import sys

for _p in ("/opt/trn_rl_repo",):
    if _p not in sys.path:
        sys.path.append(_p)

import contextlib

import numpy as np

import concourse.bass as bass
import concourse.bacc as bacc
import concourse.mybir as mybir
from concourse.tile import TileContext
from concourse import bass_utils

N, HID, ACT, NH = 256, 128, 5, 4
D, E, HD = 144, 576, 144
NCORES = 8
F32 = mybir.dt.float32
F32R = mybir.dt.float32r
SCALE = 1.0 / 12.0
CF = 145  # C' feature dim: act(128) + obs(16) + ones(1)

# blob1 [128, B1_COLS] column layout (host packing must match kernel slicing)
# dma1: G''A(145) hTp(256) wenc(16)     -> cols 0:417
# dma2: G''B(145) aTp(256)              -> cols 417:818
# dma3: mT0(256) mT1(256) WvaugA(8, padded) bT1m(1) -> cols 818:1339
B1_GA, B1_HT, B1_WENC = 0, 145, 401
B1_D1 = 417
B1_GB, B1_AT = 417, 562
B1_D2 = 818
B1_MT0, B1_MT1, B1_WVA, B1_BT1 = 818, 1074, 1330, 1338
B1_COLS = 1339
# blob2 [17, 10]: WvaugB(8, padded from 5) benc(1, rows 0:16) bT1t(1)
B2_WVB, B2_BENC, B2_BT1T = 0, 8, 9
B2_COLS = 10


def _build():
    nc = bacc.Bacc(target_bir_lowering=False)

    def dp(name, shape, dtype, isOutput=False):
        return nc.declare_dram_parameter(name, shape, dtype, isOutput)

    b1_d = dp("blob1", [128, B1_COLS], F32)
    b2_d = dp("blob2", [17, B2_COLS], F32)
    out_d = dp("out", [5, N], F32, isOutput=True)

    with TileContext(nc) as tc:
        with contextlib.ExitStack() as ctx:
            wp = ctx.enter_context(tc.tile_pool(name="wp", bufs=1))
            pp = ctx.enter_context(tc.tile_pool(name="pp", bufs=8, space="PSUM"))

            def wt(shape, tag, dtype=F32R):
                return wp.tile(shape, dtype, tag=tag, name=tag)

            def ps(shape):
                return pp.tile(shape, F32, tag="mm", name="mm")

            # ---------- DMAs: two sync-queue chunks (need order), one on
            # scalar (later-needed), blob2 via SWDGE (independent path) ----
            b1 = wt([128, B1_COLS], "b1")
            nc.sync.dma_start(out=b1[:, 0:B1_D1],
                              in_=b1_d[:, 0:B1_D1].bitcast(F32R))
            nc.sync.dma_start(out=b1[:, B1_D1:B1_D2],
                              in_=b1_d[:, B1_D1:B1_D2].bitcast(F32R))
            nc.scalar.dma_start(out=b1[:, B1_D2:B1_COLS],
                                in_=b1_d[:, B1_D2:B1_COLS].bitcast(F32R))
            b2 = wt([17, B2_COLS], "b2")
            nc.gpsimd.dma_start(out=b2, in_=b2_d[:, :].bitcast(F32R))

            GA = b1[:, B1_GA:B1_GA + CF]
            hTp = b1[:, B1_HT:B1_HT + N]
            wenc = b1[:, B1_WENC:B1_WENC + 16]
            GB = b1[:, B1_GB:B1_GB + CF]
            aTp = b1[:, B1_AT:B1_AT + N]
            mT = [b1[:, B1_MT0:B1_MT0 + N], b1[:, B1_MT1:B1_MT1 + N]]
            WvaugA = b1[:, B1_WVA:B1_WVA + 8]
            bT1m = b1[:, B1_BT1:B1_BT1 + 1].bitcast(F32)
            WvaugB = b2[:, B2_WVB:B2_WVB + 8]
            benc = b2[0:16, B2_BENC:B2_BENC + 1].bitcast(F32)
            bT1t = b2[:, B2_BT1T:B2_BT1T + 1].bitcast(F32)

            # ---------- C'^T tail tile: obs rows + ones row ----------
            ctT_full = wt([32, N], "ctT")
            nc.vector.memset(ctT_full[:, :].bitcast(F32), 1.0)
            pObs = ps([16, N])
            nc.tensor.matmul(pObs, wenc, hTp, start=True, stop=True)
            nc.vector.tensor_scalar(ctT_full[0:16, :], pObs, benc, None,
                                    mybir.AluOpType.add)
            ctT = ctT_full[0:17, :]

            # ---------- T1^T = G''^T [h;a]^T + bT1: [128,256] + [17,256] ----
            pT1m = ps([128, N])
            nc.tensor.matmul(pT1m, GA[:, 0:128], hTp, start=True, stop=False)
            nc.tensor.matmul(pT1m, GB[:, 0:128], aTp, start=False, stop=True)
            T1m = wt([128, N], "T1m")
            nc.scalar.activation(T1m, pT1m,
                                 mybir.ActivationFunctionType.Identity,
                                 bias=bT1m, scale=1.0)
            pT1t = ps([17, N])
            nc.tensor.matmul(pT1t, GA[:, 128:CF], hTp, start=True, stop=False)
            nc.tensor.matmul(pT1t, GB[:, 128:CF], aTp, start=False, stop=True)
            T1t = wt([17, N], "T1t")
            nc.vector.tensor_scalar(T1t, pT1t, bT1t, None, mybir.AluOpType.add)

            # ---------- S^T chunks -> E^T = exp(S^T/12) ----------
            ET = []
            for kc in range(2):
                pT = ps([128, 128])
                nc.tensor.matmul(pT, aTp[:, kc * 128:(kc + 1) * 128],
                                 T1m[:, 0:128], start=True, stop=False)
                nc.tensor.matmul(pT, ctT[:, kc * 128:(kc + 1) * 128],
                                 T1t[:, 0:128], start=False, stop=True)
                t = wt([128, 128], f"ET{kc}")
                nc.scalar.activation(t, pT, mybir.ActivationFunctionType.Exp,
                                     scale=SCALE)
                ET.append(t)

            # ---------- S = T1_jhalf C'^T -> E = exp(S/12) ----------
            pS = ps([128, N])
            nc.tensor.matmul(pS, T1m[:, 0:128], aTp, start=True, stop=False)
            nc.tensor.matmul(pS, T1t[:, 0:128], ctT, start=False, stop=True)
            Et = wt([128, N], "Et")
            nc.scalar.activation(Et, pS, mybir.ActivationFunctionType.Exp,
                                 scale=SCALE)

            # ---------- D; R = mask^T/max(D,eps) (approx recip) ----------
            pD = ps([128, N])
            nc.tensor.matmul(pD, ET[0], mT[0], start=True, stop=False)
            nc.tensor.matmul(pD, ET[1], mT[1], start=False, stop=True)
            Rt = wt([128, N], "Rt", F32)
            nc.vector.tensor_scalar(Rt, pD, 1e-9, None, mybir.AluOpType.max)
            Rr = wt([128, N], "Rr", F32)
            nc.vector.reciprocal_approx_fast(out=Rr, in_=Rt)
            R = wt([128, N], "R")
            nc.vector.tensor_tensor(R, Rr.bitcast(F32R), mT[0],
                                    mybir.AluOpType.mult)

            # ---------- u = C' @ Wv_aug : [128,5] x2 k-chunks ----------
            u_t = []
            for kc in range(2):
                pu = ps([128, 8])
                nc.tensor.matmul(pu, aTp[:, kc * 128:(kc + 1) * 128], WvaugA,
                                 start=True, stop=False)
                nc.tensor.matmul(pu, ctT[:, kc * 128:(kc + 1) * 128], WvaugB,
                                 start=False, stop=True)
                t = wt([128, 8], f"u{kc}")
                nc.vector.tensor_copy(out=t, in_=pu)
                u_t.append(t)

            # ---------- W = mask^T * (E^T-partial over my j-half) ----------
            Wt = []
            for kc in range(2):
                pW = ps([128, N])
                nc.tensor.matmul(pW, Et[:, kc * 128:(kc + 1) * 128], R,
                                 start=True, stop=True)
                t = wt([128, N], f"W{kc}")
                nc.vector.tensor_tensor(t, pW, mT[kc].bitcast(F32),
                                        mybir.AluOpType.mult)
                Wt.append(t)

            # ---------- partial Q^T = u^T-contract with W : [5,256] ----------
            pQ = ps([8, N])
            nc.tensor.matmul(pQ, u_t[0], Wt[0], start=True, stop=False)
            nc.tensor.matmul(pQ, u_t[1], Wt[1], start=False, stop=True)
            Qsb = wt([8, N], "Qsb", F32)
            nc.vector.tensor_copy(out=Qsb, in_=pQ)
            nc.gpsimd.dma_start(out=out_d[:, :], in_=Qsb[0:5, :])

    nc.compile()
    return nc


_NC_CACHE = {}


def _make_in_maps(inputs):
    f32 = np.float32
    g = lambda k: np.asarray(inputs[k], dtype=np.float64)

    hidden = np.asarray(inputs["hidden_state_n"], dtype=f32)
    action = np.asarray(inputs["action_n"], dtype=f32)
    state = np.asarray(inputs["state_n"]).astype(np.int64)

    # host-side weight folding (float64, cast to f32 at the end)
    Wq_eff = g("Wq") @ g("Wiq")
    bq_eff = g("bq") @ g("Wiq") + g("biq")
    Wk_eff = g("Wk") @ g("Wik")
    bk_eff = g("bk") @ g("Wik") + g("bik")
    Wv_eff = g("Wv") @ g("Wiv")
    bv_eff = g("bv") @ g("Wiv") + g("biv")
    Wo_eff = g("Wo_proj") @ g("W_O")          # [576,144]
    bo_eff = g("bo_proj") @ g("W_O")          # [144]
    W_adv = g("W_adv")
    W_Q = (g("W_val") @ np.ones((1, ACT)) + W_adv
           - (W_adv @ np.ones((ACT, ACT))) / ACT)              # [144,5]
    b_Q = g("b_val")[0] + g("b_adv") - g("b_adv").mean()       # [5]
    W_out = Wo_eff @ W_Q                                       # [576,5]
    c1 = (bo_eff @ W_Q).astype(f32)                            # [5]
    c2 = b_Q.astype(f32)                                       # [5]

    # mask from int state (host): mask[i,j] = j observed by i
    dx = np.abs(state[:, None, 0] - state[None, :, 0])
    dy = np.abs(state[:, None, 1] - state[None, :, 1])
    upper = np.arange(N)[None, :] > np.arange(N)[:, None]
    mask = ((dx <= 4) & (dy <= 2) & upper).astype(f32)         # [N,N]
    n_i = mask.sum(axis=1)                                     # [N]
    maskT = np.ascontiguousarray(mask.T)                       # [j,i]

    W_enc = g("W_enc")                                         # [128,16]
    b_enc = np.asarray(inputs["b_enc"], dtype=f32)             # [16]
    hT = np.ascontiguousarray(hidden.T)                        # [128,256]
    aT = np.ascontiguousarray(action.T)

    in_maps = []
    for c in range(NCORES):
        h, jm = c // 2, c % 2
        perm = np.roll(np.arange(N), -jm * 128)
        cols = slice(144 * h, 144 * h + 144)
        # A-mats in C'-feature row order [act(128), obs(16), ones(1)]
        def amat(W, b):
            Wh, bh = W[:, cols], b[cols]
            return np.vstack([Wh[16:144], Wh[0:16], bh[None, :]])  # [145,144]
        Aq, Ak, Av = amat(Wq_eff, bq_eff), amat(Wk_eff, bk_eff), \
            amat(Wv_eff, bv_eff)
        Gp = Aq @ Ak.T                                         # [145,145]
        GppA = W_enc @ Gp[128:144, :]                          # hid rows [128,145]
        GppB = Gp[0:128, :]                                    # act rows [128,145]
        bT1 = Gp[144, :]                                       # [145]
        Wv_aug = np.concatenate([Av @ W_out[cols, :],
                                 np.zeros((CF, 3))], axis=1)   # [145,8] padded
        mTp = maskT[perm, :]
        b1 = np.concatenate([
            GppA.astype(f32), hT[:, perm], W_enc.astype(f32),
            GppB.astype(f32), aT[:, perm],
            mTp[0:128], mTp[128:256], Wv_aug[0:128].astype(f32),
            bT1[0:128].astype(f32).reshape(128, 1)], axis=1)
        b2 = np.concatenate([
            Wv_aug[128:145].astype(f32),
            np.concatenate([b_enc.reshape(16, 1), np.zeros((1, 1), f32)]),
            bT1[128:145].astype(f32).reshape(17, 1)], axis=1)
        in_maps.append({
            "blob1": np.ascontiguousarray(b1, dtype=f32),
            "blob2": np.ascontiguousarray(b2, dtype=f32),
        })
    return in_maps, n_i, c1, c2


def kernel(**inputs):
    if "nc" not in _NC_CACHE:
        _NC_CACHE["nc"] = _build()
    nc = _NC_CACHE["nc"]
    in_maps, n_i, c1, c2 = _make_in_maps(inputs)
    res = bass_utils.run_bass_kernel_spmd(nc, in_maps, core_ids=list(range(NCORES)))
    QT = np.zeros((ACT, N), np.float32)
    for c in range(NCORES):
        QT += res.results[c]["out"]
    Q = QT.T + n_i[:, None] * c1[None, :] + c2[None, :]
    return Q.astype(np.float32)


# revision 8
# speedup vs baseline: 5.0023x; 1.0510x over previous
"""AttentionCritic Trainium2 kernel — 8-core SPMD, head/query-half sharded.

Math restructuring (exact up to fp assoc.):
  mask[i,j] = (|x_i-x_j|<=4)&(|y_i-y_j|<=2)&(j>i)          (host, from int state)
  C' = [act(128), obs(16), 1]  (ones row folds all biases)
  S_h = C' G' C'^T / 12 with G' = Aq Ak^T host-folded [145,145]
        (Aq = [Wq_eff rows | bq_eff] etc. — the two-stage reference
        projections (C@Wq+bq)@Wiq+biq are first folded to single eff mats)
  T1 = C' G' computed X-form: T1^T = G''^T [h;a]^T + bT1 (G'' absorbs the
        obs encoder for T1 only; bT1 = ones-row of G', added in the
        PSUM->SBUF copy as a per-partition bias)
  E_h = exp(S_h)  (softmax ratio is shift-invariant; |S| small enough that
        exp is safely fp32 — validated on this data by earlier versions)
  D[j,i] = sum_k E[j,k] mask[i,k];  R = mask^T/max(D,1e-9)  (approx recip)
  W[k,i] = mask[i,k] * sum_j E[j,k] R[j,i]
  u = C' @ (Av W_out_h)  [256,5]  (v/ctx/out-proj fold: Q_p = sum_k W[k,i] u[k,a])
  Q = sum_cores Q_p^T + n_i*c1 + c2  (host; W_out = Wo_eff @ Wdueling [576,5])

Sharding: core c handles (head h=c//2, query-half jm=c%2). Everything after
exp is linear in j and h, so each core emits a partial Q^T [5,256] over ALL
256 agents (free dim 256 keeps fp32r matmuls at the fast 1 cyc/row rate) and
the host sums the 8 partials. The j-half selection is uniform across cores:
the per-core input packing rotates the agent axis by 128*jm, so slice
[0:128] is always "my" j-half.

Per-core: 19 matmuls / ~3.9K stream rows, ~700KB DMA.
"""

import sys

for _p in ("/opt/trn_rl_repo",):
    if _p not in sys.path:
        sys.path.append(_p)

import contextlib

import numpy as np

import concourse.bass as bass
import concourse.bacc as bacc
import concourse.mybir as mybir
from concourse.tile import TileContext
from concourse import bass_utils

N, HID, ACT, NH = 256, 128, 5, 4
D, E, HD = 144, 576, 144
NCORES = 8
F32 = mybir.dt.float32
F32R = mybir.dt.float32r
SCALE = 1.0 / 12.0
CF = 145  # C' feature dim: act(128) + obs(16) + ones(1)

# blob1 [128, B1_COLS] column layout (host packing must match kernel slicing)
# dma1: G''A(145) hTp(256) wenc(16)     -> cols 0:417
# dma2: G''B(145) aTp(256)              -> cols 417:818
# dma3: mT0(256) mT1(256) WvaugA(8, padded) bT1m(1) id128(128) -> cols 818:1467
B1_GA, B1_HT, B1_WENC = 0, 145, 401
B1_D1 = 417
B1_GB, B1_AT = 417, 562
B1_D2 = 818
B1_MT0, B1_MT1, B1_WVA, B1_BT1 = 818, 1074, 1330, 1338
B1_ID = 1339
B1_COLS = 1467
# blob2 [17, 10]: WvaugB(8, padded from 5) benc(1, rows 0:16) bT1t(1)
B2_WVB, B2_BENC, B2_BT1T = 0, 8, 9
B2_COLS = 10


def _build():
    nc = bacc.Bacc(target_bir_lowering=False)

    def dp(name, shape, dtype, isOutput=False):
        return nc.declare_dram_parameter(name, shape, dtype, isOutput)

    b1_d = dp("blob1", [128, B1_COLS], F32)
    b2_d = dp("blob2", [17, B2_COLS], F32)
    out_d = dp("out", [5, N], F32, isOutput=True)

    with TileContext(nc) as tc:
        with contextlib.ExitStack() as ctx:
            wp = ctx.enter_context(tc.tile_pool(name="wp", bufs=1))
            pp = ctx.enter_context(tc.tile_pool(name="pp", bufs=8, space="PSUM"))

            def wt(shape, tag, dtype=F32R):
                return wp.tile(shape, dtype, tag=tag, name=tag)

            def ps(shape, dtype=F32):
                return pp.tile(shape, dtype, tag="mm", name="mm")

            # ---------- DMAs: two sync-queue chunks (need order), one on
            # scalar (later-needed), blob2 via SWDGE (independent path) ----
            b1 = wt([128, B1_COLS], "b1")
            nc.sync.dma_start(out=b1[:, 0:B1_D1],
                              in_=b1_d[:, 0:B1_D1].bitcast(F32R))
            nc.scalar.dma_start(out=b1[:, B1_D1:B1_D2],
                                in_=b1_d[:, B1_D1:B1_D2].bitcast(F32R))
            b2 = wt([17, B2_COLS], "b2")
            nc.gpsimd.dma_start(out=b2, in_=b2_d[:, :].bitcast(F32R))
            nc.gpsimd.dma_start(out=b1[:, B1_D2:B1_COLS],
                                in_=b1_d[:, B1_D2:B1_COLS].bitcast(F32R))

            GA = b1[:, B1_GA:B1_GA + CF]
            hTp = b1[:, B1_HT:B1_HT + N]
            wenc = b1[:, B1_WENC:B1_WENC + 16]
            GB = b1[:, B1_GB:B1_GB + CF]
            aTp = b1[:, B1_AT:B1_AT + N]
            mT = [b1[:, B1_MT0:B1_MT0 + N], b1[:, B1_MT1:B1_MT1 + N]]
            WvaugA = b1[:, B1_WVA:B1_WVA + 8]
            bT1m = b1[:, B1_BT1:B1_BT1 + 1].bitcast(F32)
            ident = b1[:, B1_ID:B1_ID + 128]
            WvaugB = b2[:, B2_WVB:B2_WVB + 8]
            benc = b2[0:16, B2_BENC:B2_BENC + 1].bitcast(F32)
            bT1t = b2[:, B2_BT1T:B2_BT1T + 1].bitcast(F32)

            # ---------- C'^T tail tile: obs rows + ones row ----------
            ctT_full = wt([32, N], "ctT")
            nc.vector.memset(ctT_full[:, :].bitcast(F32), 1.0)
            pObs = ps([16, N])
            nc.tensor.matmul(pObs, wenc, hTp, start=True, stop=True)
            nc.vector.tensor_scalar(ctT_full[0:16, :], pObs, benc, None,
                                    mybir.AluOpType.add)
            ctT = ctT_full[0:17, :]

            # ---------- T1^T = G''^T [h;a]^T + bT1: [128,256] + [17,256] ----
            pT1m = ps([128, N])
            nc.tensor.matmul(pT1m, GA[:, 0:128], hTp, start=True, stop=False)
            nc.tensor.matmul(pT1m, GB[:, 0:128], aTp, start=False, stop=True)
            T1m = wt([128, N], "T1m")
            nc.scalar.activation(T1m, pT1m,
                                 mybir.ActivationFunctionType.Identity,
                                 bias=bT1m, scale=1.0)
            pT1t = ps([17, N])
            nc.tensor.matmul(pT1t, GA[:, 128:CF], hTp, start=True, stop=False)
            nc.tensor.matmul(pT1t, GB[:, 128:CF], aTp, start=False, stop=True)
            T1t = wt([17, N], "T1t")
            nc.vector.tensor_scalar(T1t, pT1t, bT1t, None, mybir.AluOpType.add)

            # ---------- S = T1_jhalf C'^T -> E = exp(S/12) (2 halves) ----
            pD = ps([128, N])
            nc.vector.memset(pD, 1e-9)
            pS = ps([128, N])
            nc.tensor.matmul(pS, T1m[:, 0:128], aTp, start=True, stop=False)
            nc.tensor.matmul(pS, T1t[:, 0:128], ctT, start=False, stop=True)
            Et = wt([128, N], "Et")
            nc.scalar.activation(Et[:, 0:128], pS[:, 0:128],
                                 mybir.ActivationFunctionType.Exp, scale=SCALE)
            nc.scalar.activation(Et[:, 128:N], pS[:, 128:N],
                                 mybir.ActivationFunctionType.Exp, scale=SCALE)

            # ---------- u = C' @ Wv_aug : [128,5] x2 k-chunks ----------
            u_t = []
            for kc in range(2):
                pu = ps([128, 8])
                nc.tensor.matmul(pu, aTp[:, kc * 128:(kc + 1) * 128], WvaugA,
                                 start=True, stop=False)
                nc.tensor.matmul(pu, ctT[:, kc * 128:(kc + 1) * 128], WvaugB,
                                 start=False, stop=True)
                t = wt([128, 8], f"u{kc}")
                nc.vector.tensor_copy(out=t, in_=pu)
                u_t.append(t)

            # ---------- E^T via PE transpose (exp(S)^T == exp(S^T)) ----------
            ET = []
            for kc in range(2):
                pT = ps([128, 128], F32R)
                nc.tensor.transpose(pT, Et[:, kc * 128:(kc + 1) * 128], ident)
                t = wt([128, 128], f"ET{kc}")
                nc.vector.tensor_copy(out=t, in_=pT)
                ET.append(t)

            # ---------- D (eps preloaded); R = mask^T * approx_recip(D) ----
            nc.tensor.matmul(pD, ET[0], mT[0], start=False, stop=False,
                             skip_group_check=True)
            nc.tensor.matmul(pD, ET[1], mT[1], start=False, stop=True,
                             skip_group_check=True)
            Rr = wt([128, N], "Rr", F32)
            nc.vector.reciprocal_approx_fast(out=Rr, in_=pD)
            R = wt([128, N], "R")
            nc.vector.tensor_tensor(R, Rr.bitcast(F32R), mT[0],
                                    mybir.AluOpType.mult)

            # ---------- W = mask^T * (E^T-partial over my j-half) ----------
            Wt = []
            for kc in range(2):
                pW = ps([128, N])
                nc.tensor.matmul(pW, Et[:, kc * 128:(kc + 1) * 128], R,
                                 start=True, stop=True)
                t = wt([128, N], f"W{kc}")
                nc.vector.tensor_tensor(t, pW, mT[kc].bitcast(F32),
                                        mybir.AluOpType.mult)
                Wt.append(t)

            # ---------- partial Q^T = u^T-contract with W : [5,256] ----------
            pQ = ps([8, N])
            nc.tensor.matmul(pQ, u_t[0], Wt[0], start=True, stop=False)
            nc.tensor.matmul(pQ, u_t[1], Wt[1], start=False, stop=True)
            Qsb = wt([8, N], "Qsb", F32)
            nc.vector.tensor_copy(out=Qsb, in_=pQ)
            nc.sync.dma_start(out=out_d[:, :], in_=Qsb[0:5, :])

    nc.compile()
    return nc


_NC_CACHE = {}


def _make_in_maps(inputs):
    f32 = np.float32
    g = lambda k: np.asarray(inputs[k], dtype=np.float64)

    hidden = np.asarray(inputs["hidden_state_n"], dtype=f32)
    action = np.asarray(inputs["action_n"], dtype=f32)
    state = np.asarray(inputs["state_n"]).astype(np.int64)

    # host-side weight folding (float64, cast to f32 at the end)
    Wq_eff = g("Wq") @ g("Wiq")
    bq_eff = g("bq") @ g("Wiq") + g("biq")
    Wk_eff = g("Wk") @ g("Wik")
    bk_eff = g("bk") @ g("Wik") + g("bik")
    Wv_eff = g("Wv") @ g("Wiv")
    bv_eff = g("bv") @ g("Wiv") + g("biv")
    Wo_eff = g("Wo_proj") @ g("W_O")          # [576,144]
    bo_eff = g("bo_proj") @ g("W_O")          # [144]
    W_adv = g("W_adv")
    W_Q = (g("W_val") @ np.ones((1, ACT)) + W_adv
           - (W_adv @ np.ones((ACT, ACT))) / ACT)              # [144,5]
    b_Q = g("b_val")[0] + g("b_adv") - g("b_adv").mean()       # [5]
    W_out = Wo_eff @ W_Q                                       # [576,5]
    c1 = (bo_eff @ W_Q).astype(f32)                            # [5]
    c2 = b_Q.astype(f32)                                       # [5]

    # mask from int state (host): mask[i,j] = j observed by i
    dx = np.abs(state[:, None, 0] - state[None, :, 0])
    dy = np.abs(state[:, None, 1] - state[None, :, 1])
    upper = np.arange(N)[None, :] > np.arange(N)[:, None]
    mask = ((dx <= 4) & (dy <= 2) & upper).astype(f32)         # [N,N]
    n_i = mask.sum(axis=1)                                     # [N]
    maskT = np.ascontiguousarray(mask.T)                       # [j,i]

    W_enc = g("W_enc")                                         # [128,16]
    b_enc = np.asarray(inputs["b_enc"], dtype=f32)             # [16]
    hT = np.ascontiguousarray(hidden.T)                        # [128,256]
    aT = np.ascontiguousarray(action.T)

    in_maps = []
    for c in range(NCORES):
        h, jm = c // 2, c % 2
        perm = np.roll(np.arange(N), -jm * 128)
        cols = slice(144 * h, 144 * h + 144)
        # A-mats in C'-feature row order [act(128), obs(16), ones(1)]
        def amat(W, b):
            Wh, bh = W[:, cols], b[cols]
            return np.vstack([Wh[16:144], Wh[0:16], bh[None, :]])  # [145,144]
        Aq, Ak, Av = amat(Wq_eff, bq_eff), amat(Wk_eff, bk_eff), \
            amat(Wv_eff, bv_eff)
        Gp = Aq @ Ak.T                                         # [145,145]
        GppA = W_enc @ Gp[128:144, :]                          # hid rows [128,145]
        GppB = Gp[0:128, :]                                    # act rows [128,145]
        bT1 = Gp[144, :]                                       # [145]
        Wv_aug = np.concatenate([Av @ W_out[cols, :],
                                 np.zeros((CF, 3))], axis=1)   # [145,8] padded
        mTp = maskT[perm, :]
        b1 = np.concatenate([
            GppA.astype(f32), hT[:, perm], W_enc.astype(f32),
            GppB.astype(f32), aT[:, perm],
            mTp[0:128], mTp[128:256], Wv_aug[0:128].astype(f32),
            bT1[0:128].astype(f32).reshape(128, 1),
            np.eye(128, dtype=f32)], axis=1)
        b2 = np.concatenate([
            Wv_aug[128:145].astype(f32),
            np.concatenate([b_enc.reshape(16, 1), np.zeros((1, 1), f32)]),
            bT1[128:145].astype(f32).reshape(17, 1)], axis=1)
        in_maps.append({
            "blob1": np.ascontiguousarray(b1, dtype=f32),
            "blob2": np.ascontiguousarray(b2, dtype=f32),
        })
    return in_maps, n_i, c1, c2


def kernel(**inputs):
    if "nc" not in _NC_CACHE:
        _NC_CACHE["nc"] = _build()
    nc = _NC_CACHE["nc"]
    in_maps, n_i, c1, c2 = _make_in_maps(inputs)
    res = bass_utils.run_bass_kernel_spmd(nc, in_maps, core_ids=list(range(NCORES)))
    QT = np.zeros((ACT, N), np.float32)
    for c in range(NCORES):
        QT += res.results[c]["out"]
    Q = QT.T + n_i[:, None] * c1[None, :] + c2[None, :]
    return Q.astype(np.float32)
